# revision 14
# baseline (speedup 1.0000x reference)
"""Batch K-means epoch kernel for Trainium2 (8 NeuronCores, data-parallel over N).

Reference computation:
    d2[n,k]   = |x_n|^2 + |c_k|^2 - 2 x_n.c_k
    dists     = sqrt(d2)
    buckets   = argmin_k dists
    new_bins  = bins + segment_sum(x, buckets)
    new_nums  = nums + bincount(buckets)
    mean_dist = mean(dists)

Kernel strategy (per core, N_shard = 16384 rows = 128 tiles of 128):
    v[n,k] = 2 x_n.c_k  computed as a 3-pass float32r matmul (hi + residuals,
    ~fp32 precision at full PE rate).  argmin_k d2 == argmax_k (v - |c|^2).
    DVE tensor_tensor_reduce forms v_sb = v - |c|^2 and its row max; max_index
    gives the argmax (first occurrence, matching jnp.argmin tie-break).
    GPSIMD builds the one-hot A; PE accumulates bins += A_chunk^T @ x (f32r).
    ACT computes x_sq (Square+accum) and sqrt(x_sq - v) row-sums for the mean.
    nums comes from a host-side bincount of the returned indices.

Host pre-work: round x/codebook to f32r hi+residual parts, pre-transpose the
x parts (the v matmul contracts over D so its stationary operand is x^T),
precompute 2*codebook^T parts and -|c|^2.
"""
import sys, os

sys.path.insert(0, "/opt/trn_rl_repo")
os.environ.setdefault("MYCRO_LOCAL_CACHE", "1")

import numpy as np

N, K, D = 131072, 1024, 256
NCORES = 8
NSH = N // NCORES  # 16384 rows per core
P = 128

_cache = {}


def round_f32r(a):
    """fp32 -> fp32r (RNE to 11-bit mantissa, low 12 bits zeroed)."""
    u = np.ascontiguousarray(a, dtype=np.float32).view(np.uint32)
    lsb = (u >> np.uint32(12)) & np.uint32(1)
    r = (u + np.uint32(0x7FF) + lsb) & np.uint32(0xFFFFF000)
    return r.view(np.float32)


def build_kernel(nsh=NSH):
    import concourse.bass as bass
    import concourse.bacc as bacc
    import concourse.mybir as mybir
    import concourse.tile as tile

    f32 = mybir.dt.float32
    f32r = mybir.dt.float32r
    u16 = mybir.dt.uint16
    NT = nsh // P

    nc = bacc.Bacc(None, target_bir_lowering=False)
    # row-major rounded x (bins matmul rhs + x_sq source)
    xh_d = nc.declare_dram_parameter("xh", [nsh, D], f32r, isOutput=False)
    # transposed x parts (v matmul stationary operands), [D, nsh]
    xth_d = nc.declare_dram_parameter("xth", [D, nsh], f32r, isOutput=False)
    xtr_d = nc.declare_dram_parameter("xtr", [D, nsh], f32r, isOutput=False)
    # 2*codebook^T parts [D, K] and -|c|^2 [1, K]
    cth_d = nc.declare_dram_parameter("cth", [D, K], f32r, isOutput=False)
    ctr_d = nc.declare_dram_parameter("ctr", [D, K], f32r, isOutput=False)
    ncsq_d = nc.declare_dram_parameter("ncsq", [1, K], f32, isOutput=False)

    bins_d = nc.declare_dram_parameter("bins_out", [D, K], f32, isOutput=True)  # bins^T
    idx_d = nc.declare_dram_parameter("idx_out", [P, NT * 8], u16, isOutput=True)
    dsum_d = nc.declare_dram_parameter("dsum_out", [P, NT], f32, isOutput=True)

    with tile.TileContext(nc) as tc:
        with (
            tc.tile_pool(name="const", bufs=1) as const,
            tc.tile_pool(name="xin", bufs=4) as xin,
            tc.tile_pool(name="xt", bufs=3) as xtp,
            tc.tile_pool(name="vsb", bufs=2) as vsbp,
            tc.tile_pool(name="dsc", bufs=2) as dscp,
            tc.tile_pool(name="apool", bufs=3) as apool,
            tc.tile_pool(name="small", bufs=4) as small,
            tc.tile_pool(name="vps", bufs=2, space="PSUM") as vpsp,
            tc.tile_pool(name="binsps", bufs=1, space="PSUM") as binsps,
        ):
            # ---- constants ----
            # D-chunks of (2 c^T): [P, 2, K] = partition x (chunk, K)
            cth_sb = const.tile([P, 2, K], f32r)
            ctr_sb = const.tile([P, 2, K], f32r)
            for dc in range(2):
                nc.sync.dma_start(out=cth_sb[:, dc, :], in_=cth_d[dc * P:(dc + 1) * P, :])
                nc.sync.dma_start(out=ctr_sb[:, dc, :], in_=ctr_d[dc * P:(dc + 1) * P, :])
            ncsq_rep = const.tile([P, K], f32)  # -|c_k|^2 replicated
            nc.sync.dma_start(out=ncsq_rep[:], in_=ncsq_d[:].to_broadcast([P, K]))
            iota_sb = const.tile([P, K], f32)
            nc.gpsimd.iota(iota_sb[:], pattern=[[1, K]], base=0,
                           channel_multiplier=0,
                           allow_small_or_imprecise_dtypes=True)
            m8 = const.tile([P, 8], f32)
            nc.vector.memset(m8[:], 1e30)

            # ---- persistent accumulators ----
            dsum_sb = const.tile([P, NT], f32)
            idx8 = const.tile([P, NT * 8], u16)
            bins_ps = [binsps.tile([P, K], f32, tag=f"bins{i}", name=f"bins_ps{i}")
                       for i in range(2)]  # bins^T D-chunks, 2 banks each

            for t in range(NT):
                row = slice(t * P, (t + 1) * P)
                x_t = xin.tile([P, D], f32r)
                nc.sync.dma_start(out=x_t[:], in_=xh_d[row, :])
                # transposed slices: [P, 2, P] = partition x (D-chunk, n)
                xt_h = xtp.tile([P, 2, P], f32r, tag="xt_h")
                xt_r = xtp.tile([P, 2, P], f32r, tag="xt_r")
                for dc in range(2):
                    nc.sync.dma_start(out=xt_h[:, dc, :], in_=xth_d[dc * P:(dc + 1) * P, row])
                    nc.sync.dma_start(out=xt_r[:, dc, :], in_=xtr_d[dc * P:(dc + 1) * P, row])

                # x_sq on ACT (Square + accumulate); f32r bits read as fp32
                sq_scr = small.tile([P, D], f32, tag="sqscr")
                xsq = small.tile([P, 1], f32, tag="xsq")
                nc.scalar.activation(out=sq_scr[:], in_=x_t[:].bitcast(f32),
                                     func=mybir.ActivationFunctionType.Square,
                                     accum_out=xsq[:, 0:1])

                # v = 2 x.c : 3-pass f32r accumulation into psum [128, 1024]
                v_ps = vpsp.tile([P, K], f32)  # 2 banks
                for kc in range(2):
                    sl = slice(kc * 512, (kc + 1) * 512)
                    passes = [(xt_h, cth_sb), (xt_r, cth_sb), (xt_h, ctr_sb)]
                    for pi, (xa, ca) in enumerate(passes):
                        for dc in range(2):
                            nc.tensor.matmul(v_ps[:, sl], xa[:, dc, :], ca[:, dc, sl],
                                             start=(pi == 0 and dc == 0),
                                             stop=(pi == 2 and dc == 1))

                # v_sb = v - |c|^2 ; row max -> m8[:,0]
                # (tensor_tensor_reduce would fuse these but crashes at runtime)
                v_sb = vsbp.tile([P, K], f32)
                nc.vector.tensor_tensor(out=v_sb[:], in0=v_ps[:], in1=ncsq_rep[:],
                                        op=mybir.AluOpType.add)
                nc.vector.tensor_reduce(out=m8[:, 0:1], in_=v_sb[:],
                                        axis=mybir.AxisListType.X,
                                        op=mybir.AluOpType.max)

                # dists row-sums on ACT: sqrt(x_sq - v_sb) = sqrt(d2), accum col t
                d_scr = dscp.tile([P, K], f32)
                nc.scalar.activation(out=d_scr[:], in_=v_sb[:],
                                     func=mybir.ActivationFunctionType.Sqrt,
                                     bias=xsq[:, 0:1], scale=-1.0,
                                     accum_out=dsum_sb[:, t:t + 1])

                # argmax index (first occurrence == jnp argmin tie-break)
                nc.vector.max_index(idx8[:, t * 8:(t + 1) * 8], m8[:], v_sb[:])

                # one-hot A on gpsimd
                idxf = small.tile([P, 1], f32, tag="idxf")
                nc.gpsimd.tensor_copy(idxf[:, 0:1], idx8[:, t * 8:t * 8 + 1])
                a_t = apool.tile([P, K], f32r)
                nc.gpsimd.tensor_scalar(out=a_t[:], in0=iota_sb[:],
                                        scalar1=idxf[:, 0:1], scalar2=None,
                                        op0=mybir.AluOpType.is_equal)

                # bins^T += x_chunk^T @ A  (contract over the 128 rows)
                for dc in range(2):
                    for kc in range(2):
                        sl = slice(kc * 512, (kc + 1) * 512)
                        nc.tensor.matmul(
                            bins_ps[dc][:, sl],
                            x_t[:, dc * P:(dc + 1) * P],
                            a_t[:, sl],
                            start=(t == 0), stop=(t == NT - 1))

            # ---- outputs ----
            bins_sb = const.tile([P, 2 * K], f32)
            for dc in range(2):
                nc.scalar.copy(bins_sb[:, dc * K:(dc + 1) * K], bins_ps[dc][:])
                nc.sync.dma_start(out=bins_d[dc * P:(dc + 1) * P, :],
                                  in_=bins_sb[:, dc * K:(dc + 1) * K])
            nc.sync.dma_start(out=dsum_d[:], in_=dsum_sb[:])
            nc.sync.dma_start(out=idx_d[:], in_=idx8[:])

    nc.finalize()
    return nc


def _get_nc(nsh=NSH):
    if nsh not in _cache:
        _cache[nsh] = build_kernel(nsh)
    return _cache[nsh]


def host_prep(x, codebook):
    """Precompute all f32r operand arrays on the host."""
    x = x.astype(np.float32)
    c = codebook.astype(np.float32)
    xh = round_f32r(x)                                   # [N, D]
    xres = round_f32r((x - xh).astype(np.float32))       # [N, D]
    ct2 = (2.0 * c.T).astype(np.float32)                 # [D, K]
    cth = round_f32r(ct2)
    ctr = round_f32r((ct2 - cth).astype(np.float32))
    ncsq = -(c * c).sum(axis=1, dtype=np.float32).astype(np.float32)[None, :]
    xth = np.ascontiguousarray(xh.T)                     # [D, N]
    xtr = np.ascontiguousarray(xres.T)                   # [D, N]
    return xh, xth, xtr, cth, ctr, ncsq


def _run(x, codebook, bins, nums, trace=False):
    from concourse.bass_utils import run_bass_kernel_spmd

    x = np.asarray(x, dtype=np.float32)
    codebook = np.asarray(codebook, dtype=np.float32)
    bins = np.asarray(bins, dtype=np.float32)
    nums = np.asarray(nums, dtype=np.float32)

    nc = _get_nc()
    xh, xth, xtr, cth, ctr, ncsq = host_prep(x, codebook)
    in_maps = []
    for i in range(NCORES):
        rs = slice(i * NSH, (i + 1) * NSH)
        in_maps.append({
            "xh": np.ascontiguousarray(xh[rs]),
            "xth": np.ascontiguousarray(xth[:, rs]),
            "xtr": np.ascontiguousarray(xtr[:, rs]),
            "cth": cth, "ctr": ctr, "ncsq": ncsq,
        })
    out = run_bass_kernel_spmd(nc, in_maps, list(range(NCORES)), trace=trace)
    res = out.results

    NT = NSH // P
    bins_total = np.zeros((K, D), dtype=np.float64)
    counts = np.zeros(K, dtype=np.int64)
    dtot = 0.0
    for r in res:
        bins_total += r["bins_out"].astype(np.float64).T
        idx = r["idx_out"].reshape(P, NT, 8)[:, :, 0]  # [P(row-in-tile), NT]
        counts += np.bincount(idx.astype(np.int64).ravel(), minlength=K)
        dtot += r["dsum_out"].astype(np.float64).sum()

    mean_dist = np.float32(dtot / (N * K))
    new_bins = (bins.astype(np.float64) + bins_total).astype(np.float32)
    new_nums = (nums.astype(np.float64) + counts[:, None]).astype(np.float32)
    extra = {"exec_time_ns": out.exec_time_ns,
             "mean_exec_time_ns": out.mean_exec_time_ns}
    return mean_dist, new_bins, new_nums, extra


def kernel(x, codebook, bins, nums):
    md, b, n, _ = _run(x, codebook, bins, nums, trace=False)
    return md, b, n


def kernel_traced(x, codebook, bins, nums):
    return _run(x, codebook, bins, nums, trace=True)


# revision 21
# speedup vs baseline: 135.5527x; 135.5527x over previous
"""Batch K-means epoch kernel for Trainium2 (8 NeuronCores, data-parallel over N).

Reference computation:
    d2[n,k]   = |x_n|^2 + |c_k|^2 - 2 x_n.c_k
    dists     = sqrt(d2)
    buckets   = argmin_k dists
    new_bins  = bins + segment_sum(x, buckets)
    new_nums  = nums + bincount(buckets)
    mean_dist = mean(dists)

Kernel strategy (per core, N_shard = 16384 rows = 128 tiles of 128):
    v[n,k] = 2 x_n.c_k  computed as a 3-pass float32r matmul (hi + residuals,
    ~fp32 precision at full PE rate).  argmin_k d2 == argmax_k (v - |c|^2).
    DVE tensor_tensor_reduce forms v_sb = v - |c|^2 and its row max; max_index
    gives the argmax (first occurrence, matching jnp.argmin tie-break).
    GPSIMD builds the one-hot A; PE accumulates bins += A_chunk^T @ x (f32r).
    ACT computes x_sq (Square+accum) and sqrt(x_sq - v) row-sums for the mean.
    nums comes from a host-side bincount of the returned indices.

Host pre-work: round x/codebook to f32r hi+residual parts, pre-transpose the
x parts (the v matmul contracts over D so its stationary operand is x^T),
precompute 2*codebook^T parts and -|c|^2.
"""
import sys, os

sys.path.insert(0, "/opt/trn_rl_repo")
os.environ.setdefault("MYCRO_LOCAL_CACHE", "1")

import numpy as np

N, K, D = 131072, 1024, 256
NCORES = 8
NSH = N // NCORES  # 16384 rows per core
P = 128

_cache = {}


def round_f32r(a):
    """fp32 -> fp32r (RNE to 11-bit mantissa, low 12 bits zeroed)."""
    u = np.ascontiguousarray(a, dtype=np.float32).view(np.uint32)
    lsb = (u >> np.uint32(12)) & np.uint32(1)
    r = (u + np.uint32(0x7FF) + lsb) & np.uint32(0xFFFFF000)
    return r.view(np.float32)


def build_kernel(nsh=NSH):
    import concourse.bass as bass
    import concourse.bacc as bacc
    import concourse.mybir as mybir
    import concourse.tile as tile

    f32 = mybir.dt.float32
    f32r = mybir.dt.float32r
    u16 = mybir.dt.uint16
    NT = nsh // P

    nc = bacc.Bacc(None, target_bir_lowering=False)
    # row-major rounded x (bins matmul rhs + x_sq source)
    xh_d = nc.declare_dram_parameter("xh", [nsh, D], f32r, isOutput=False)
    # transposed x parts (v matmul stationary operands), [D, nsh]
    xth_d = nc.declare_dram_parameter("xth", [D, nsh], f32r, isOutput=False)
    xtr_d = nc.declare_dram_parameter("xtr", [D, nsh], f32r, isOutput=False)
    # 2*codebook^T parts [D, K] and -|c|^2 [1, K]
    cth_d = nc.declare_dram_parameter("cth", [D, K], f32r, isOutput=False)
    ctr_d = nc.declare_dram_parameter("ctr", [D, K], f32r, isOutput=False)
    ncsq_d = nc.declare_dram_parameter("ncsq", [1, K], f32, isOutput=False)

    bins_d = nc.declare_dram_parameter("bins_out", [D, K], f32, isOutput=True)  # bins^T
    idx_d = nc.declare_dram_parameter("idx_out", [P, NT * 8], u16, isOutput=True)
    dsum_d = nc.declare_dram_parameter("dsum_out", [P, NT], f32, isOutput=True)

    with tile.TileContext(nc) as tc:
        with (
            tc.tile_pool(name="const", bufs=1) as const,
            tc.tile_pool(name="xin", bufs=6) as xin,
            tc.tile_pool(name="xt", bufs=4) as xtp,
            tc.tile_pool(name="vsb", bufs=3) as vsbp,
            tc.tile_pool(name="dsc", bufs=3) as dscp,
            tc.tile_pool(name="apool", bufs=4) as apool,
            tc.tile_pool(name="small", bufs=4) as small,
            tc.tile_pool(name="vps", bufs=2, space="PSUM") as vpsp,
            tc.tile_pool(name="binsps", bufs=1, space="PSUM") as binsps,
        ):
            # ---- constants ----
            # D-chunks of (2 c^T): [P, 2, K] = partition x (chunk, K)
            cth_sb = const.tile([P, 2, K], f32r)
            ctr_sb = const.tile([P, 2, K], f32r)
            for dc in range(2):
                nc.sync.dma_start(out=cth_sb[:, dc, :], in_=cth_d[dc * P:(dc + 1) * P, :])
                nc.sync.dma_start(out=ctr_sb[:, dc, :], in_=ctr_d[dc * P:(dc + 1) * P, :])
            ncsq_rep = const.tile([P, K], f32)  # -|c_k|^2 replicated
            nc.sync.dma_start(out=ncsq_rep[:], in_=ncsq_d[:].to_broadcast([P, K]))
            iota_sb = const.tile([P, K], f32)
            nc.gpsimd.iota(iota_sb[:], pattern=[[1, K]], base=0,
                           channel_multiplier=0,
                           allow_small_or_imprecise_dtypes=True)
            m8 = const.tile([P, 8], f32)
            nc.vector.memset(m8[:], 1e30)

            # ---- persistent accumulators ----
            dsum_sb = const.tile([P, NT], f32)
            idx8 = const.tile([P, NT * 8], u16)
            bins_ps = [binsps.tile([P, K], f32, tag=f"bins{i}", name=f"bins_ps{i}")
                       for i in range(2)]  # bins^T D-chunks, 2 banks each

            for t in range(NT):
                row = slice(t * P, (t + 1) * P)
                x_t = xin.tile([P, D], f32r)
                nc.sync.dma_start(out=x_t[:], in_=xh_d[row, :])
                # transposed slices: [P, 2, P] = partition x (D-chunk, n)
                xt_h = xtp.tile([P, 2, P], f32r, tag="xt_h")
                xt_r = xtp.tile([P, 2, P], f32r, tag="xt_r")
                for dc in range(2):
                    nc.sync.dma_start(out=xt_h[:, dc, :], in_=xth_d[dc * P:(dc + 1) * P, row])
                    nc.sync.dma_start(out=xt_r[:, dc, :], in_=xtr_d[dc * P:(dc + 1) * P, row])

                # x_sq on ACT (Square + accumulate); f32r bits read as fp32
                sq_scr = small.tile([P, D], f32, tag="sqscr")
                xsq = small.tile([P, 1], f32, tag="xsq")
                nc.scalar.activation(out=sq_scr[:], in_=x_t[:].bitcast(f32),
                                     func=mybir.ActivationFunctionType.Square,
                                     accum_out=xsq[:, 0:1])

                # v = 2 x.c : 3-pass f32r accumulation into psum [128, 1024]
                v_ps = vpsp.tile([P, K], f32)  # 2 banks
                for kc in range(2):
                    sl = slice(kc * 512, (kc + 1) * 512)
                    passes = [(xt_h, cth_sb), (xt_r, cth_sb), (xt_h, ctr_sb)]
                    for pi, (xa, ca) in enumerate(passes):
                        for dc in range(2):
                            nc.tensor.matmul(v_ps[:, sl], xa[:, dc, :], ca[:, dc, sl],
                                             start=(pi == 0 and dc == 0),
                                             stop=(pi == 2 and dc == 1))

                # v_sb = v - |c|^2 ; row max -> m8[:,0]
                # (tensor_tensor_reduce would fuse these but crashes at runtime)
                v_sb = vsbp.tile([P, K], f32)
                nc.vector.tensor_tensor(out=v_sb[:], in0=v_ps[:], in1=ncsq_rep[:],
                                        op=mybir.AluOpType.add)
                nc.vector.tensor_reduce(out=m8[:, 0:1], in_=v_sb[:],
                                        axis=mybir.AxisListType.X,
                                        op=mybir.AluOpType.max)

                # dists row-sums on ACT: sqrt(x_sq - v_sb) = sqrt(d2), accum col t
                d_scr = dscp.tile([P, K], f32)
                nc.scalar.activation(out=d_scr[:], in_=v_sb[:],
                                     func=mybir.ActivationFunctionType.Sqrt,
                                     bias=xsq[:, 0:1], scale=-1.0,
                                     accum_out=dsum_sb[:, t:t + 1])

                # argmax index (first occurrence == jnp argmin tie-break)
                nc.vector.max_index(idx8[:, t * 8:(t + 1) * 8], m8[:], v_sb[:])

                # one-hot A on DVE (f32 iota vs cast idx; scalar1 must be f32)
                idxf = small.tile([P, 1], f32, tag="idxf")
                nc.scalar.copy(idxf[:, 0:1], idx8[:, t * 8:t * 8 + 1])
                a_t = apool.tile([P, K], f32r)
                nc.vector.tensor_scalar(out=a_t[:], in0=iota_sb[:],
                                        scalar1=idxf[:, 0:1], scalar2=None,
                                        op0=mybir.AluOpType.is_equal)

                # bins^T += x_chunk^T @ A  (contract over the 128 rows)
                for dc in range(2):
                    for kc in range(2):
                        sl = slice(kc * 512, (kc + 1) * 512)
                        nc.tensor.matmul(
                            bins_ps[dc][:, sl],
                            x_t[:, dc * P:(dc + 1) * P],
                            a_t[:, sl],
                            start=(t == 0), stop=(t == NT - 1))

            # ---- outputs ----
            bins_sb = const.tile([P, 2 * K], f32)
            for dc in range(2):
                nc.scalar.copy(bins_sb[:, dc * K:(dc + 1) * K], bins_ps[dc][:])
                nc.sync.dma_start(out=bins_d[dc * P:(dc + 1) * P, :],
                                  in_=bins_sb[:, dc * K:(dc + 1) * K])
            nc.sync.dma_start(out=dsum_d[:], in_=dsum_sb[:])
            nc.sync.dma_start(out=idx_d[:], in_=idx8[:])

    nc.finalize()
    return nc


def _get_nc(nsh=NSH):
    if nsh not in _cache:
        _cache[nsh] = build_kernel(nsh)
    return _cache[nsh]


def host_prep(x, codebook):
    """Precompute all f32r operand arrays on the host."""
    x = x.astype(np.float32)
    c = codebook.astype(np.float32)
    xh = round_f32r(x)                                   # [N, D]
    xres = round_f32r((x - xh).astype(np.float32))       # [N, D]
    ct2 = (2.0 * c.T).astype(np.float32)                 # [D, K]
    cth = round_f32r(ct2)
    ctr = round_f32r((ct2 - cth).astype(np.float32))
    ncsq = -(c * c).sum(axis=1, dtype=np.float32).astype(np.float32)[None, :]
    xth = np.ascontiguousarray(xh.T)                     # [D, N]
    xtr = np.ascontiguousarray(xres.T)                   # [D, N]
    return xh, xth, xtr, cth, ctr, ncsq


def _run(x, codebook, bins, nums, trace=False, tmpdir=None):
    from concourse.bass_utils import run_bass_kernel_spmd

    x = np.asarray(x, dtype=np.float32)
    codebook = np.asarray(codebook, dtype=np.float32)
    bins = np.asarray(bins, dtype=np.float32)
    nums = np.asarray(nums, dtype=np.float32)

    nc = _get_nc()
    xh, xth, xtr, cth, ctr, ncsq = host_prep(x, codebook)
    in_maps = []
    for i in range(NCORES):
        rs = slice(i * NSH, (i + 1) * NSH)
        in_maps.append({
            "xh": np.ascontiguousarray(xh[rs]),
            "xth": np.ascontiguousarray(xth[:, rs]),
            "xtr": np.ascontiguousarray(xtr[:, rs]),
            "cth": cth, "ctr": ctr, "ncsq": ncsq,
        })
    kw = {"tmpdir": tmpdir} if tmpdir else {}
    out = run_bass_kernel_spmd(nc, in_maps, list(range(NCORES)), trace=trace, **kw)
    res = out.results

    NT = NSH // P
    bins_total = np.zeros((K, D), dtype=np.float64)
    counts = np.zeros(K, dtype=np.int64)
    dtot = 0.0
    for r in res:
        bins_total += r["bins_out"].astype(np.float64).T
        idx = r["idx_out"].reshape(P, NT, 8)[:, :, 0]  # [P(row-in-tile), NT]
        counts += np.bincount(idx.astype(np.int64).ravel(), minlength=K)
        dtot += r["dsum_out"].astype(np.float64).sum()

    mean_dist = np.float32(dtot / (N * K))
    new_bins = (bins.astype(np.float64) + bins_total).astype(np.float32)
    new_nums = (nums.astype(np.float64) + counts[:, None]).astype(np.float32)
    extra = {"exec_time_ns": out.exec_time_ns,
             "mean_exec_time_ns": out.mean_exec_time_ns}
    return mean_dist, new_bins, new_nums, extra


def kernel(x, codebook, bins, nums):
    md, b, n, _ = _run(x, codebook, bins, nums, trace=False)
    return md, b, n


def kernel_traced(x, codebook, bins, nums):
    return _run(x, codebook, bins, nums, trace=True)
